# revision 42
# baseline (speedup 1.0000x reference)
"""Trainium2 Bass kernel for a custom LSTM cell.

Math (per reference):
    i = sigmoid(x @ W_i.T + b_Wi + h @ U_i.T + b_Ui)
    f = sigmoid(x @ W_f.T + b_Wf + h @ U_f.T + b_Uf + boundary @ W_b.T + b_Wb)
    o = sigmoid(x @ W_o.T + b_Wo + h @ U_o.T + b_Uo)
    g = tanh   (x @ W_g.T + b_Wg + h @ U_g.T + b_Ug)
    c = f * c_prev + i * g
    h = o * tanh(c)

Strategy: data-parallel over batch across 8 NeuronCores (1024 rows each),
computed TRANSPOSED on-device: hidden on partitions, batch on the free axis.
With hidden on partitions the gate biases become per-partition ACT-engine
bias operands (free), and the boundary term is a K=2 matmul accumulated
straight into the f-gate PSUM group — no K=3 bias matmuls.

Matmul operands are bf16 (well within the 2e-2 error budget), halving HBM
traffic vs f32/f32r. Per h-slice of 128 hidden rows the gates run in two
waves (i,g then f,o) of [128,512] PSUM tiles so the 8 PSUM banks hold two
(slice, batch-half) units in flight and the PE never waits on drains.
"""

import sys

sys.path.insert(0, "/opt/trn_rl_repo")

import numpy as np
import ml_dtypes

BF16 = ml_dtypes.bfloat16

B, IN, H = 8192, 512, 1024
NCORES = 8
BLOC = B // NCORES  # 1024 batch rows per core
KTOT = IN + H  # 1536 contraction
KT = KTOT // 128  # 12 k-tiles
NS = H // 128  # 8 h-slices of 128 hidden rows
GW = 4 * 128  # 512 columns of M per h-slice (i|g|f|o)
HALF = BLOC // 2  # 512-wide batch halves (one PSUM bank each)

_PROG = None  # cached so repeat calls skip rebuild/recompile


def _build_program():
    import concourse.mybir as mybir
    import concourse.tile as tile
    from concourse import bacc
    from contextlib import ExitStack

    f32 = mybir.dt.float32
    bf = mybir.dt.bfloat16
    SIG = mybir.ActivationFunctionType.Sigmoid
    TANH = mybir.ActivationFunctionType.Tanh

    nc = bacc.Bacc("TRN2", target_bir_lowering=False, debug=False)

    a_d = nc.dram_tensor("a_in", [KTOT, BLOC], bf, kind="ExternalInput").ap()
    m_d = nc.dram_tensor("m_in", [KTOT, 4 * H], bf, kind="ExternalInput").ap()
    bias_d = nc.dram_tensor("bias_in", [128, 4 * NS], f32, kind="ExternalInput").ap()
    bdi_d = nc.dram_tensor("bdi_in", [H, BLOC], f32, kind="ExternalInput").ap()
    ct_d = nc.dram_tensor("ct_in", [H, BLOC], f32, kind="ExternalInput").ap()
    ht_o = nc.dram_tensor("ht_out", [H, BLOC], f32, kind="ExternalOutput").ap()
    ct_o = nc.dram_tensor("ct_out", [H, BLOC], f32, kind="ExternalOutput").ap()

    with tile.TileContext(nc) as tc:
        with ExitStack() as ctx:
            apl = ctx.enter_context(tc.tile_pool(name="apl", bufs=1))
            mp = ctx.enter_context(tc.tile_pool(name="mp", bufs=3))
            cst = ctx.enter_context(tc.tile_pool(name="cst", bufs=1))
            ctp = ctx.enter_context(tc.tile_pool(name="ctp", bufs=4))
            gp = ctx.enter_context(tc.tile_pool(name="gp", bufs=6))
            ep = ctx.enter_context(tc.tile_pool(name="ep", bufs=4))
            outp = ctx.enter_context(tc.tile_pool(name="outp", bufs=4))
            psp = ctx.enter_context(tc.tile_pool(name="psp", bufs=8, space="PSUM"))
            wup = ctx.enter_context(tc.tile_pool(name="wup", bufs=1))

            # Small PE warm-up: absorbs the p-state ramp while the first
            # activation/weight chunks land.
            wu_w = wup.tile([128, 128], bf, name="wu_w")
            nc.vector.memset(wu_w, 0.0)
            wu_ps = psp.tile([128, 512], f32, name="wu_ps", tag="ps")
            for _ in range(40):
                nc.tensor.matmul(wu_ps[:, 0:128], wu_w, wu_w, start=True, stop=True)

            bias_t = cst.tile([128, 4 * NS], f32, name="bias_t")
            nc.scalar.dma_start(out=bias_t, in_=bias_d[:, :])

            def load_m_slice(s):
                """[128, 12, 512] weight tile for h-slice s, 3 big 3D DMAs."""
                t = mp.tile([128, KT, GW], bf, name=f"m_{s}", tag="m")
                for j in range(3):
                    nc.sync.dma_start(
                        out=t[:, j * 4 : (j + 1) * 4, :],
                        in_=m_d[
                            j * 512 : (j + 1) * 512, s * GW : (s + 1) * GW
                        ].rearrange("(kk p) g -> p kk g", p=128),
                    )
                return t

            def load_ct_half(s, h2, eng=None):
                t = ctp.tile([128, HALF], f32, name=f"ct_{s}_{h2}", tag="ct")
                (eng or nc.scalar).dma_start(
                    out=t,
                    in_=ct_d[
                        s * 128 : (s + 1) * 128, h2 * HALF : (h2 + 1) * HALF
                    ],
                )
                return t

            def load_bdi_half(s, h2, eng=None):
                t = ctp.tile([128, HALF], f32, name=f"bdi_{s}_{h2}", tag="bdi")
                (eng or nc.scalar).dma_start(
                    out=t,
                    in_=bdi_d[
                        s * 128 : (s + 1) * 128, h2 * HALF : (h2 + 1) * HALF
                    ],
                )
                return t

            # A and slice-0 weights land as separate kk=2 chunk tiles so each
            # matmul pair only waits on its own 0.75MB, not the whole slice.
            # The two startup queues (sync, gpsimd) round-robin for bandwidth,
            # so each queue carries one element of every (a_j, m0_j) pair —
            # that keeps delivery in need-order with both queues busy.
            # chunk j covers k-tiles CH_LO[j]..CH_LO[j]+CH_KK[j]-1; the first
            # two are single-k so the opening matmul's dependency is minimal
            CH_KK = [1, 1, 2, 2, 2, 2, 2]
            CH_LO = [0, 1, 2, 4, 6, 8, 10]
            K2CH = [2, 3, 4, 5, 6, 7]  # filled properly below
            K2CH = []
            for j, (lo, kk) in enumerate(zip(CH_LO, CH_KK)):
                K2CH += [(j, k - lo) for k in range(lo, lo + kk)]
            a_ts = []
            m0_ts = []
            for j, (lo, kk) in enumerate(zip(CH_LO, CH_KK)):
                at = apl.tile([128, kk, BLOC], bf, name=f"a_t{j}")
                a_eng = nc.sync if j % 2 == 0 else nc.gpsimd
                m_eng = nc.gpsimd if j % 2 == 0 else nc.sync
                a_eng.dma_start(
                    out=at,
                    in_=a_d[lo * 128 : (lo + kk) * 128, :].rearrange(
                        "(kk p) b -> p kk b", p=128
                    ),
                )
                mt = apl.tile([128, kk, GW], bf, name=f"m0_t{j}")
                m_eng.dma_start(
                    out=mt,
                    in_=m_d[lo * 128 : (lo + kk) * 128, 0:GW].rearrange(
                        "(kk p) g -> p kk g", p=128
                    ),
                )
                a_ts.append(at)
                m0_ts.append(mt)
            # slice-1 weights stream right behind the startup chunks — unit
            # (1,h0)'s matmuls need them the moment unit (0,h0) finishes. The
            # c_prev/boundary halves can wait: drains slip a few us hidden
            # under the next unit's matmuls.
            m1_ts = []
            for j in range(6):
                mt = apl.tile([128, 2, GW], bf, name=f"m1_t{j}")
                eng = nc.sync if j % 2 == 0 else nc.gpsimd
                eng.dma_start(
                    out=mt,
                    in_=m_d[
                        j * 256 : (j + 1) * 256, GW : 2 * GW
                    ].rearrange("(kk p) g -> p kk g", p=128),
                )
                m1_ts.append(mt)
            ct00 = load_ct_half(0, 0, eng=nc.gpsimd)
            bdi00 = load_bdi_half(0, 0, eng=nc.sync)
            ct10 = load_ct_half(1, 0, eng=nc.gpsimd)
            bdi10 = load_bdi_half(1, 0, eng=nc.sync)
            ct01 = load_ct_half(0, 1, eng=nc.gpsimd)
            bdi01 = load_bdi_half(0, 1, eng=nc.sync)
            ct11 = load_ct_half(1, 1, eng=nc.gpsimd)
            bdi11 = load_bdi_half(1, 1, eng=nc.sync)

            def a_ap(k, bs):
                j, o = K2CH[k]
                return a_ts[j][:, o, bs]

            def gate_acts(s, h2, pss, cth, bdih):
                """Activations + elementwise + stores for one (s, h2) unit.
                cth/bdih are per-half [128, 512] tiles (local columns)."""
                b0 = 4 * s
                bs = slice(h2 * HALF, (h2 + 1) * HALF)
                ps_i, ps_g, ps_f, ps_o = pss["i"], pss["g"], pss["f"], pss["o"]
                i_t = gp.tile([128, HALF], f32, name=f"i{s}_{h2}", tag="g")
                g_t = gp.tile([128, HALF], f32, name=f"g{s}_{h2}", tag="g")
                nc.scalar.activation(i_t, ps_i, SIG, bias=bias_t[:, b0 : b0 + 1])
                nc.scalar.activation(g_t, ps_g, TANH, bias=bias_t[:, b0 + 1 : b0 + 2])
                ig_t = ep.tile([128, HALF], f32, name=f"ig{s}_{h2}", tag="ig")
                nc.vector.tensor_mul(ig_t, i_t, g_t)
                f_t = gp.tile([128, HALF], f32, name=f"f{s}_{h2}", tag="g")
                o_t = gp.tile([128, HALF], f32, name=f"o{s}_{h2}", tag="g")

                # boundary influence lands in the f-gate PSUM via one DVE add
                # (saves a K=2 matmul in the PE stream per unit)
                nc.vector.tensor_add(ps_f, ps_f, bdih)

                # c' = f*c_prev + i*g ; h = o*tanh(c'). The very last unit
                # runs in 256-wide chunks to shorten the serial tail chain.
                hs = slice(s * 128, (s + 1) * 128)
                cn = outp.tile([128, HALF], f32, name=f"cn{s}_{h2}", tag="cn")
                th = ep.tile([128, HALF], f32, name=f"th{s}_{h2}", tag="th")
                hn = outp.tile([128, HALF], f32, name=f"hn{s}_{h2}", tag="hn")
                last = s == NS - 1 and h2 == 1
                for q0, q1 in ([(0, 256), (256, HALF)] if last else [(0, HALF)]):
                    qs = slice(q0, q1)
                    nc.scalar.activation(
                        f_t[:, qs], ps_f[:, qs], SIG, bias=bias_t[:, b0 + 2 : b0 + 3]
                    )
                    nc.scalar.activation(
                        o_t[:, qs], ps_o[:, qs], SIG, bias=bias_t[:, b0 + 3 : b0 + 4]
                    )
                    nc.vector.tensor_mul(cn[:, qs], f_t[:, qs], cth[:, qs])
                    nc.vector.tensor_add(cn[:, qs], cn[:, qs], ig_t[:, qs])
                    nc.scalar.activation(th[:, qs], cn[:, qs], TANH)
                    nc.vector.tensor_mul(hn[:, qs], o_t[:, qs], th[:, qs])
                nc.gpsimd.dma_start(out=ct_o[hs, bs], in_=cn)
                nc.gpsimd.dma_start(out=ht_o[hs, bs], in_=hn)

            # Slices 0 and 1 are supply-limited (A plus their weights stream
            # in during the first ~25us), so their four (s, h2) units run in
            # an order that maximizes PE work per supplied byte:
            # (0,h0) -> (1,h0) -> (0,h1) -> (1,h1).
            def m0_ap(k, c0, c1):
                j, o = K2CH[k]
                return m0_ts[j][:, o, c0:c1]

            def m1_ap(k, c0, c1):
                return m1_ts[k // 2][:, k % 2, c0:c1]

            def mm_unit(s, h2, m_ap, pads=False):
                pss = {
                    z: psp.tile([128, HALF], f32, name=f"ps{z}{s}_{h2}", tag="ps")
                    for z in "igfo"
                }
                bs = slice(h2 * HALF, (h2 + 1) * HALF)
                for k in range(KT):
                    st = k == 0
                    sp = k == KT - 1
                    rhs = a_ap(k, bs)
                    nc.tensor.matmul(pss["i"], m_ap(k, 0, 128), rhs, start=st, stop=sp)
                    nc.tensor.matmul(
                        pss["g"], m_ap(k, 128, 256), rhs, start=st, stop=sp
                    )
                    nc.tensor.matmul(
                        pss["f"], m_ap(k, 256, 384), rhs, start=st, stop=sp
                    )
                    nc.tensor.matmul(
                        pss["o"], m_ap(k, 384, 512), rhs, start=st, stop=sp
                    )
                    # pace-matching pad: consumption slightly outruns the DMA
                    # supply; a zero-weight accumulate (adds exactly 0) keeps
                    # the PE continuously busy so its p-state never resets.
                    if pads and k % 2 == 1 and k < KT - 1:
                        for _ in range(3):
                            nc.tensor.matmul(
                                pss["i"][:, 0:256], wu_w, a_ts[0][:, 0, 0:256],
                                start=False, stop=False,
                            )
                return pss

            ps00 = mm_unit(0, 0, m0_ap, pads=True)
            ps10 = mm_unit(1, 0, m1_ap)
            gate_acts(0, 0, ps00, ct00, bdi00)
            ps01 = mm_unit(0, 1, m0_ap)
            gate_acts(1, 0, ps10, ct10, bdi10)
            ps11 = mm_unit(1, 1, m1_ap)
            gate_acts(0, 1, ps01, ct01, bdi01)
            gate_acts(1, 1, ps11, ct11, bdi11)

            for s in range(2, NS):
                m_t = load_m_slice(s)
                ct_hs = [load_ct_half(s, 0), load_ct_half(s, 1)]
                bdi_hs = [load_bdi_half(s, 0), load_bdi_half(s, 1)]
                for h2 in range(2):
                    bs = slice(h2 * HALF, (h2 + 1) * HALF)
                    # wave 1: i, g
                    ps_i = psp.tile([128, HALF], f32, name=f"psi{s}_{h2}", tag="ps")
                    ps_g = psp.tile([128, HALF], f32, name=f"psg{s}_{h2}", tag="ps")
                    for k in range(KT):
                        rhs = a_ap(k, bs)
                        nc.tensor.matmul(
                            ps_i, m_t[:, k, 0:128], rhs,
                            start=(k == 0), stop=(k == KT - 1),
                        )
                        nc.tensor.matmul(
                            ps_g, m_t[:, k, 128:256], rhs,
                            start=(k == 0), stop=(k == KT - 1),
                        )
                    # wave 2: f, o
                    ps_f = psp.tile([128, HALF], f32, name=f"psf{s}_{h2}", tag="ps")
                    ps_o = psp.tile([128, HALF], f32, name=f"pso{s}_{h2}", tag="ps")
                    for k in range(KT):
                        rhs = a_ap(k, bs)
                        nc.tensor.matmul(
                            ps_f, m_t[:, k, 256:384], rhs,
                            start=(k == 0), stop=(k == KT - 1),
                        )
                        nc.tensor.matmul(
                            ps_o, m_t[:, k, 384:512], rhs,
                            start=(k == 0), stop=(k == KT - 1),
                        )
                    gate_acts(
                        s, h2,
                        {"i": ps_i, "g": ps_g, "f": ps_f, "o": ps_o},
                        ct_hs[h2], bdi_hs[h2],
                    )
    nc.compile()
    return nc


def _get_program():
    global _PROG
    if _PROG is None:
        _PROG = _build_program()
    return _PROG


def _prep_inputs(inputs):
    """Host-side marshalling: fused bf16 weight matrix + transposed acts."""
    f = np.float32
    x = np.asarray(inputs["x"], f)
    h_prev = np.asarray(inputs["h_prev"], f)
    c_prev = np.asarray(inputs["c_prev"], f)
    boundary = np.asarray(inputs["boundary"], f)

    gates = ["i", "g", "f", "o"]
    W = {z: np.asarray(inputs[f"W_{z}"], f) for z in gates}
    U = {z: np.asarray(inputs[f"U_{z}"], f) for z in gates}
    bias = {
        z: np.asarray(inputs[f"b_W{z}"], f) + np.asarray(inputs[f"b_U{z}"], f)
        for z in gates
    }
    W_b = np.asarray(inputs["W_b"], f)
    b_Wb = np.asarray(inputs["b_Wb"], f)
    bias["f"] = bias["f"] + b_Wb

    # M [1536, 4096]: rows 0-511 W.T, rows 512-1535 U.T; columns grouped per
    # 128-wide h-slice as [i | g | f | o].
    M = np.empty((KTOT, 4 * H), f)
    BIAS = np.empty((128, 4 * NS), f)
    for s in range(NS):
        hs = slice(s * 128, (s + 1) * 128)
        for gi, z in enumerate(gates):
            cs = slice(s * GW + gi * 128, s * GW + (gi + 1) * 128)
            M[:IN, cs] = W[z][hs].T
            M[IN:, cs] = U[z][hs].T
            BIAS[:, 4 * s + gi] = bias[z][hs]

    Mb = M.astype(BF16)
    AT = np.concatenate([x, h_prev], axis=1).T  # [1536, 8192] f32
    ATb = np.ascontiguousarray(AT).astype(BF16)
    # boundary influence (minus its bias, already folded into BIAS) computed
    # host-side: [B, H] -> transposed per-core slices like c_prev
    BDI = (boundary @ W_b.T).astype(f)  # [8192, 1024]

    in_maps = []
    for c in range(NCORES):
        rs = slice(c * BLOC, (c + 1) * BLOC)
        in_maps.append(
            {
                "a_in": np.ascontiguousarray(ATb[:, rs]),
                "m_in": Mb,
                "bias_in": BIAS,
                "bdi_in": np.ascontiguousarray(BDI[rs].T),
                "ct_in": np.ascontiguousarray(c_prev[rs].T),
            }
        )
    return in_maps


def run(inputs, trace=False):
    """Returns ((h, c), BassKernelResults)."""
    from concourse.bass_utils import run_bass_kernel_spmd

    nc = _get_program()
    in_maps = _prep_inputs(inputs)
    res = run_bass_kernel_spmd(
        nc, in_maps, core_ids=list(range(NCORES)), trace=trace
    )
    h = np.concatenate(
        [np.ascontiguousarray(r["ht_out"].T) for r in res.results], axis=0
    )
    c = np.concatenate(
        [np.ascontiguousarray(r["ct_out"].T) for r in res.results], axis=0
    )
    return (h, c), res


def kernel(**inputs):
    out, _ = run(inputs, trace=False)
    return out


# revision 43
# speedup vs baseline: 1.0379x; 1.0379x over previous
"""Trainium2 Bass kernel for a custom LSTM cell.

Math (per reference):
    i = sigmoid(x @ W_i.T + b_Wi + h @ U_i.T + b_Ui)
    f = sigmoid(x @ W_f.T + b_Wf + h @ U_f.T + b_Uf + boundary @ W_b.T + b_Wb)
    o = sigmoid(x @ W_o.T + b_Wo + h @ U_o.T + b_Uo)
    g = tanh   (x @ W_g.T + b_Wg + h @ U_g.T + b_Ug)
    c = f * c_prev + i * g
    h = o * tanh(c)

Strategy: data-parallel over batch across 8 NeuronCores (1024 rows each),
computed TRANSPOSED on-device: hidden on partitions, batch on the free axis.
With hidden on partitions the gate biases become per-partition ACT-engine
bias operands (free), and the boundary term (precomputed host-side as
boundary @ W_b.T) is added into the f-gate PSUM with one DVE op — the PE
stream is pure gate matmuls.

Matmul operands are bf16 (well within the 2e-2 error budget), halving HBM
traffic vs f32/f32r. Per h-slice of 128 hidden rows the gates run in two
waves (i,g then f,o) of [128,512] PSUM tiles so the 8 PSUM banks hold two
(slice, batch-half) units in flight and the PE never waits on drains.
Slice 0 is supply-limited (A + its weights stream in during the first
~14us), so it runs all 8 accumulators in one k-major pass whose chunked
DMA dependencies match the delivery order.
"""

import sys

sys.path.insert(0, "/opt/trn_rl_repo")

import numpy as np
import ml_dtypes

BF16 = ml_dtypes.bfloat16

B, IN, H = 8192, 512, 1024
NCORES = 8
BLOC = B // NCORES  # 1024 batch rows per core
KTOT = IN + H  # 1536 contraction
KT = KTOT // 128  # 12 k-tiles
NS = H // 128  # 8 h-slices of 128 hidden rows
GW = 4 * 128  # 512 columns of M per h-slice (i|g|f|o)
HALF = BLOC // 2  # 512-wide batch halves (one PSUM bank each)

_PROG = None  # cached so repeat calls skip rebuild/recompile


def _build_program():
    import concourse.mybir as mybir
    import concourse.tile as tile
    from concourse import bacc
    from contextlib import ExitStack

    f32 = mybir.dt.float32
    bf = mybir.dt.bfloat16
    SIG = mybir.ActivationFunctionType.Sigmoid
    TANH = mybir.ActivationFunctionType.Tanh

    nc = bacc.Bacc("TRN2", target_bir_lowering=False, debug=False)

    a_d = nc.dram_tensor("a_in", [KTOT, BLOC], bf, kind="ExternalInput").ap()
    m_d = nc.dram_tensor("m_in", [KTOT, 4 * H], bf, kind="ExternalInput").ap()
    bias_d = nc.dram_tensor("bias_in", [128, 4 * NS], f32, kind="ExternalInput").ap()
    bdi_d = nc.dram_tensor("bdi_in", [H, BLOC], f32, kind="ExternalInput").ap()
    ct_d = nc.dram_tensor("ct_in", [H, BLOC], f32, kind="ExternalInput").ap()
    ht_o = nc.dram_tensor("ht_out", [H, BLOC], f32, kind="ExternalOutput").ap()
    ct_o = nc.dram_tensor("ct_out", [H, BLOC], f32, kind="ExternalOutput").ap()

    with tile.TileContext(nc) as tc:
        with ExitStack() as ctx:
            apl = ctx.enter_context(tc.tile_pool(name="apl", bufs=1))
            mp = ctx.enter_context(tc.tile_pool(name="mp", bufs=3))
            cst = ctx.enter_context(tc.tile_pool(name="cst", bufs=1))
            ctp = ctx.enter_context(tc.tile_pool(name="ctp", bufs=2))
            gp = ctx.enter_context(tc.tile_pool(name="gp", bufs=6))
            ep = ctx.enter_context(tc.tile_pool(name="ep", bufs=4))
            outp = ctx.enter_context(tc.tile_pool(name="outp", bufs=4))
            psp = ctx.enter_context(tc.tile_pool(name="psp", bufs=8, space="PSUM"))
            wup = ctx.enter_context(tc.tile_pool(name="wup", bufs=1))

            # Small PE warm-up: absorbs the p-state ramp while the first
            # activation/weight chunks land.
            wu_w = wup.tile([128, 128], bf, name="wu_w")
            nc.vector.memset(wu_w, 0.0)
            wu_ps = psp.tile([128, 512], f32, name="wu_ps", tag="ps")
            for _ in range(40):
                nc.tensor.matmul(wu_ps[:, 0:128], wu_w, wu_w, start=True, stop=True)

            bias_t = cst.tile([128, 4 * NS], f32, name="bias_t")
            nc.scalar.dma_start(out=bias_t, in_=bias_d[:, :])

            def load_m_slice(s):
                """[128, 12, 512] weight tile for h-slice s, 3 big 3D DMAs."""
                t = mp.tile([128, KT, GW], bf, name=f"m_{s}", tag="m")
                for j in range(3):
                    nc.sync.dma_start(
                        out=t[:, j * 4 : (j + 1) * 4, :],
                        in_=m_d[
                            j * 512 : (j + 1) * 512, s * GW : (s + 1) * GW
                        ].rearrange("(kk p) g -> p kk g", p=128),
                    )
                return t

            def load_ct_slice(s, eng=None):
                t = ctp.tile([128, BLOC], f32, name=f"ct_{s}", tag="ct")
                (eng or nc.scalar).dma_start(
                    out=t, in_=ct_d[s * 128 : (s + 1) * 128, :]
                )
                return t

            def load_bdi_slice(s, eng=None):
                t = ctp.tile([128, BLOC], f32, name=f"bdi_{s}", tag="bdi")
                (eng or nc.scalar).dma_start(
                    out=t, in_=bdi_d[s * 128 : (s + 1) * 128, :]
                )
                return t

            # A and slice-0 weights land as separate kk=2 chunk tiles so each
            # matmul pair only waits on its own 0.75MB, not the whole slice.
            # A issues on the sync queue, slice-0 weights on the (otherwise
            # idle at startup) gpsimd queue so the ~0.7us per-issue costs
            # overlap.
            a_ts = []
            m0_ts = []
            for j in range(6):
                at = apl.tile([128, 2, BLOC], bf, name=f"a_t{j}")
                nc.sync.dma_start(
                    out=at,
                    in_=a_d[j * 256 : (j + 1) * 256, :].rearrange(
                        "(kk p) b -> p kk b", p=128
                    ),
                )
                a_ts.append(at)
            for j in range(6):
                mt = apl.tile([128, 2, GW], bf, name=f"m0_t{j}")
                nc.gpsimd.dma_start(
                    out=mt,
                    in_=m_d[j * 256 : (j + 1) * 256, 0:GW].rearrange(
                        "(kk p) g -> p kk g", p=128
                    ),
                )
                m0_ts.append(mt)
            # slice-0 c_prev rides the scalar queue (small), boundary behind
            # the A chunks on sync: both are only needed at the slice-0 drain.
            ct_t = load_ct_slice(0)
            bdi_t = load_bdi_slice(0, eng=nc.sync)

            def a_ap(k, bs):
                return a_ts[k // 2][:, k % 2, bs]

            def gate_acts(s, h2, ps_i, ps_g, ps_f, ps_o, ct_t, bdi_t):
                """Activations + elementwise + stores for one (s, h2) unit."""
                b0 = 4 * s
                bs = slice(h2 * HALF, (h2 + 1) * HALF)
                i_t = gp.tile([128, HALF], f32, name=f"i{s}_{h2}", tag="g")
                g_t = gp.tile([128, HALF], f32, name=f"g{s}_{h2}", tag="g")
                nc.scalar.activation(i_t, ps_i, SIG, bias=bias_t[:, b0 : b0 + 1])
                nc.scalar.activation(g_t, ps_g, TANH, bias=bias_t[:, b0 + 1 : b0 + 2])
                ig_t = ep.tile([128, HALF], f32, name=f"ig{s}_{h2}", tag="ig")
                nc.vector.tensor_mul(ig_t, i_t, g_t)
                f_t = gp.tile([128, HALF], f32, name=f"f{s}_{h2}", tag="g")
                o_t = gp.tile([128, HALF], f32, name=f"o{s}_{h2}", tag="g")

                # c' = f*c_prev + i*g ; h = o*tanh(c'). The very last unit
                # runs in 256-wide chunks to shorten the serial tail chain.
                hs = slice(s * 128, (s + 1) * 128)
                cn = outp.tile([128, HALF], f32, name=f"cn{s}_{h2}", tag="cn")
                th = ep.tile([128, HALF], f32, name=f"th{s}_{h2}", tag="th")
                hn = outp.tile([128, HALF], f32, name=f"hn{s}_{h2}", tag="hn")
                last = s == NS - 1 and h2 == 1
                for q0, q1 in ([(0, 256), (256, HALF)] if last else [(0, HALF)]):
                    qs = slice(q0, q1)
                    bqs = slice(h2 * HALF + q0, h2 * HALF + q1)
                    # boundary influence lands in the f-gate PSUM via one DVE
                    # add (saves a K=2 matmul in the PE stream per unit)
                    nc.vector.tensor_add(
                        ps_f[:, qs], ps_f[:, qs], bdi_t[:, bqs]
                    )
                    nc.scalar.activation(
                        f_t[:, qs], ps_f[:, qs], SIG, bias=bias_t[:, b0 + 2 : b0 + 3]
                    )
                    nc.scalar.activation(
                        o_t[:, qs], ps_o[:, qs], SIG, bias=bias_t[:, b0 + 3 : b0 + 4]
                    )
                    nc.vector.tensor_mul(cn[:, qs], f_t[:, qs], ct_t[:, bqs])
                    nc.vector.tensor_add(cn[:, qs], cn[:, qs], ig_t[:, qs])
                    nc.scalar.activation(th[:, qs], cn[:, qs], TANH)
                    nc.vector.tensor_mul(hn[:, qs], o_t[:, qs], th[:, qs])
                nc.gpsimd.dma_start(out=ct_o[hs, bs], in_=cn)
                # the final h store issues on the scalar queue so its ~0.65us
                # issue cost overlaps the c store's instead of serializing
                (nc.scalar if last else nc.gpsimd).dma_start(
                    out=ht_o[hs, bs], in_=hn
                )

            # Slice 0 is supply-limited (A + its weights stream in during the
            # first ~14us): run both batch halves' 8 accumulators in one pass,
            # h2 interleaved inside k, so PE consumption per chunk stays
            # behind the DMA supply.
            ps0 = {}
            for h2 in range(2):
                for z in "igfo":
                    ps0[z, h2] = psp.tile(
                        [128, HALF], f32, name=f"ps{z}0_{h2}", tag="ps"
                    )

            def m0_ap(k, c0, c1):
                return m0_ts[k // 2][:, k % 2, c0:c1]

            for k in range(KT):
                st = k == 0
                sp = k == KT - 1
                for h2 in range(2):
                    bs = slice(h2 * HALF, (h2 + 1) * HALF)
                    rhs = a_ap(k, bs)
                    nc.tensor.matmul(
                        ps0["i", h2], m0_ap(k, 0, 128), rhs, start=st, stop=sp
                    )
                    nc.tensor.matmul(
                        ps0["g", h2], m0_ap(k, 128, 256), rhs, start=st, stop=sp
                    )
                    nc.tensor.matmul(
                        ps0["f", h2], m0_ap(k, 256, 384), rhs, start=st, stop=sp
                    )
                    nc.tensor.matmul(
                        ps0["o", h2], m0_ap(k, 384, 512), rhs, start=st, stop=sp
                    )
            for h2 in range(2):
                gate_acts(
                    0, h2, ps0["i", h2], ps0["g", h2], ps0["f", h2], ps0["o", h2],
                    ct_t, bdi_t,
                )

            for s in range(1, NS):
                m_t = load_m_slice(s)
                ct_t = load_ct_slice(s)
                bdi_t = load_bdi_slice(s)
                for h2 in range(2):
                    bs = slice(h2 * HALF, (h2 + 1) * HALF)
                    # wave 1: i, g
                    ps_i = psp.tile([128, HALF], f32, name=f"psi{s}_{h2}", tag="ps")
                    ps_g = psp.tile([128, HALF], f32, name=f"psg{s}_{h2}", tag="ps")
                    for k in range(KT):
                        rhs = a_ap(k, bs)
                        nc.tensor.matmul(
                            ps_i, m_t[:, k, 0:128], rhs,
                            start=(k == 0), stop=(k == KT - 1),
                        )
                        nc.tensor.matmul(
                            ps_g, m_t[:, k, 128:256], rhs,
                            start=(k == 0), stop=(k == KT - 1),
                        )
                    # wave 2: f, o
                    ps_f = psp.tile([128, HALF], f32, name=f"psf{s}_{h2}", tag="ps")
                    ps_o = psp.tile([128, HALF], f32, name=f"pso{s}_{h2}", tag="ps")
                    for k in range(KT):
                        rhs = a_ap(k, bs)
                        nc.tensor.matmul(
                            ps_f, m_t[:, k, 256:384], rhs,
                            start=(k == 0), stop=(k == KT - 1),
                        )
                        nc.tensor.matmul(
                            ps_o, m_t[:, k, 384:512], rhs,
                            start=(k == 0), stop=(k == KT - 1),
                        )
                    gate_acts(s, h2, ps_i, ps_g, ps_f, ps_o, ct_t, bdi_t)
    nc.compile()
    return nc


def _get_program():
    global _PROG
    if _PROG is None:
        _PROG = _build_program()
    return _PROG


def _prep_inputs(inputs):
    """Host-side marshalling: fused bf16 weight matrix + transposed acts."""
    f = np.float32
    x = np.asarray(inputs["x"], f)
    h_prev = np.asarray(inputs["h_prev"], f)
    c_prev = np.asarray(inputs["c_prev"], f)
    boundary = np.asarray(inputs["boundary"], f)

    gates = ["i", "g", "f", "o"]
    W = {z: np.asarray(inputs[f"W_{z}"], f) for z in gates}
    U = {z: np.asarray(inputs[f"U_{z}"], f) for z in gates}
    bias = {
        z: np.asarray(inputs[f"b_W{z}"], f) + np.asarray(inputs[f"b_U{z}"], f)
        for z in gates
    }
    W_b = np.asarray(inputs["W_b"], f)
    b_Wb = np.asarray(inputs["b_Wb"], f)
    bias["f"] = bias["f"] + b_Wb

    # M [1536, 4096]: rows 0-511 W.T, rows 512-1535 U.T; columns grouped per
    # 128-wide h-slice as [i | g | f | o].
    M = np.empty((KTOT, 4 * H), f)
    BIAS = np.empty((128, 4 * NS), f)
    for s in range(NS):
        hs = slice(s * 128, (s + 1) * 128)
        for gi, z in enumerate(gates):
            cs = slice(s * GW + gi * 128, s * GW + (gi + 1) * 128)
            M[:IN, cs] = W[z][hs].T
            M[IN:, cs] = U[z][hs].T
            BIAS[:, 4 * s + gi] = bias[z][hs]

    Mb = M.astype(BF16)
    AT = np.concatenate([x, h_prev], axis=1).T  # [1536, 8192] f32
    ATb = np.ascontiguousarray(AT).astype(BF16)
    # boundary influence (minus its bias, already folded into BIAS) computed
    # host-side: [B, H] -> transposed per-core slices like c_prev
    BDI = (boundary @ W_b.T).astype(f)  # [8192, 1024]

    in_maps = []
    for c in range(NCORES):
        rs = slice(c * BLOC, (c + 1) * BLOC)
        in_maps.append(
            {
                "a_in": np.ascontiguousarray(ATb[:, rs]),
                "m_in": Mb,
                "bias_in": BIAS,
                "bdi_in": np.ascontiguousarray(BDI[rs].T),
                "ct_in": np.ascontiguousarray(c_prev[rs].T),
            }
        )
    return in_maps


def run(inputs, trace=False):
    """Returns ((h, c), BassKernelResults)."""
    from concourse.bass_utils import run_bass_kernel_spmd

    nc = _get_program()
    in_maps = _prep_inputs(inputs)
    res = run_bass_kernel_spmd(
        nc, in_maps, core_ids=list(range(NCORES)), trace=trace
    )
    h = np.concatenate(
        [np.ascontiguousarray(r["ht_out"].T) for r in res.results], axis=0
    )
    c = np.concatenate(
        [np.ascontiguousarray(r["ct_out"].T) for r in res.results], axis=0
    )
    return (h, c), res


def kernel(**inputs):
    out, _ = run(inputs, trace=False)
    return out
